# revision 49
# baseline (speedup 1.0000x reference)
"""Multi-head self-attention (RoPE, causal) on 8 trn2 NeuronCores.

Sharding: batch (4) x head-group (2x8 heads) = 8 shards, one per core.
Each core: QKV projection for its 8 heads -> RoPE -> causal flash
attention (scores kept transposed [k, q]; softmax denominators
accumulated on the PE via ones-column matmuls) -> partial o_proj over
its 512 head-dims.  Host sums the two partial o_proj outputs of each
batch pair (the tensor-parallel all-reduce) and concatenates batches.

v2 restructure vs baseline:
- causal mask applied post-exp as a bf16 0/1 multiply on SBUF (DVE 2x
  mode) instead of fp32 PSUM adds.
- softmax reciprocal via reciprocal_approx_fast (5x faster than the
  iterative-divide reciprocal).
- 1/denom broadcast across the 64 head rows via gpsimd
  partition_broadcast instead of PE ones-matmuls + DVE copy.
- rope final add moved to the gpsimd (Pool) engine.
- attention outputs (aT) stay resident in SBUF as bf16 -- no DRAM
  round-trip before o_proj; w_o cast to bf16 (FWL weight loads).
- o_proj tiles for token-block tb are interleaved into head-pair 3's
  attention as soon as aT[:, :, tb] is complete.
- V projection split into per-token-block units interleaved into
  head-pair 0's attention; xT DMA'd in (tb, d) chunks so compute
  starts early.
- diagonal score matmuls trimmed to the needed query range (>=256 to
  keep fp32r at full rate).
"""
import sys
import math

sys.path.insert(0, "/opt/trn_rl_repo")

import numpy as np
from contextlib import ExitStack

import concourse.bacc as bacc
import concourse.tile as tile
from concourse import mybir
from concourse.bass_utils import run_bass_kernel_spmd

B, S, D, H, DK = 4, 2048, 1024, 16, 64
NCORES = 8
ND = D // 128          # 8 d-tiles of the model dim
NT = S // 512          # 4 token super-blocks
NKT = S // 128         # 16 key/token 128-blocks
HPC = H // 2           # heads per core = 8
NHP = HPC // 2         # head-pairs per core = 4
F32 = mybir.dt.float32
F32R = mybir.dt.float32r
BF16 = mybir.dt.bfloat16

_CACHE = {}


def _build():
    nc = bacc.Bacc("TRN2", target_bir_lowering=False, num_devices=NCORES)

    xT_d = nc.dram_tensor("xT", [D, S], BF16, kind="ExternalInput")
    wq_d = nc.dram_tensor("wq", [D, HPC * DK], BF16, kind="ExternalInput")
    wk_d = nc.dram_tensor("wk", [D, HPC * DK], BF16, kind="ExternalInput")
    wv_d = nc.dram_tensor("wv", [D, HPC * DK], BF16, kind="ExternalInput")
    wo_d = nc.dram_tensor("wo", [HPC * DK, D], BF16, kind="ExternalInput")
    ropeC_d = nc.dram_tensor("ropeC", [128, S], BF16, kind="ExternalInput")
    ropeS_d = nc.dram_tensor("ropeS", [128, S], BF16, kind="ExternalInput")
    mask_d = nc.dram_tensor("mask", [128, 2, 128], BF16, kind="ExternalInput")
    yT_d = nc.dram_tensor("yT", [D, S], BF16, kind="ExternalOutput")

    with ExitStack() as ctx:
        tc = ctx.enter_context(tile.TileContext(nc))

        const = ctx.enter_context(tc.tile_pool(name="const", bufs=1))
        # PSUM pools: score 2banks x2 + proj 1bank x2 + po 2banks = 8
        ps = ctx.enter_context(tc.tile_pool(name="ps", bufs=2, space="PSUM"))
        pv = ctx.enter_context(tc.tile_pool(name="pv", bufs=1, space="PSUM"))

        # ---- constants (DMAs for rope emitted later, after hot inputs) --
        ropeC = const.tile([128, S], BF16)
        ropeS = const.tile([128, S], BF16)
        mask01 = const.tile([128, 2, 128], BF16)
        ones_f = const.tile([128, 1], F32)
        nc.vector.memset(ones_f, 1.0)
        ones_col = const.tile([128, 1], BF16)
        nc.vector.tensor_copy(ones_col, ones_f)
        ones_row_f = const.tile([65, 64], F32)
        nc.vector.memset(ones_row_f, 1.0)
        ones_row = const.tile([65, 64], BF16)
        nc.vector.tensor_copy(ones_row, ones_row_f)

        with ExitStack() as phase_a:
            xpool = phase_a.enter_context(tc.tile_pool(name="x", bufs=1))
            vpool = phase_a.enter_context(tc.tile_pool(name="v", bufs=1))
            qkpool = phase_a.enter_context(tc.tile_pool(name="qk", bufs=2))
            wpool = phase_a.enter_context(tc.tile_pool(name="w", bufs=2))
            tmp = phase_a.enter_context(tc.tile_pool(name="tmp", bufs=2))
            es = phase_a.enter_context(tc.tile_pool(name="es", bufs=6))
            apool = phase_a.enter_context(tc.tile_pool(name="a", bufs=1))
            wvpool = phase_a.enter_context(tc.tile_pool(name="wv", bufs=1))
            wopool = phase_a.enter_context(tc.tile_pool(name="wo", bufs=1))
            ypool = phase_a.enter_context(tc.tile_pool(name="y", bufs=2))

            # ---- V projection weights first (V units need them first) --
            # d-halves so the first projection matmuls can start before
            # the full tile has landed
            wv_sb = wvpool.tile([128, ND, HPC * DK], BF16)
            xT = xpool.tile([128, ND, S], BF16)

            def wv_half(dh):
                nc.sync.dma_start(
                    out=wv_sb[:, 4 * dh : 4 * (dh + 1), :],
                    in_=wv_d[512 * dh : 512 * (dh + 1), :].rearrange(
                        "(d p) c -> p d c", p=128
                    ),
                )

            def xT_chunk(tb, split=False, only_dh=None):
                cs = slice(512 * tb, 512 * (tb + 1))
                for dh in range(2 if split else 1):
                    if only_dh is not None and dh != only_dh:
                        continue
                    ds = slice(4 * dh, 4 * (dh + 1)) if split else slice(0, ND)
                    rs = (
                        slice(512 * dh, 512 * (dh + 1))
                        if split
                        else slice(0, D)
                    )
                    nc.sync.dma_start(
                        out=xT[:, ds, cs],
                        in_=xT_d[rs, cs].rearrange("(d p) s -> p d s", p=128),
                    )

            # first-compute critical path: wv/xT0 halves interleaved
            wv_half(0)
            xT_chunk(0, split=True, only_dh=0)
            wv_half(1)
            xT_chunk(0, split=True, only_dh=1)
            nc.gpsimd.dma_start(out=mask01[:, :, :], in_=mask_d[:, :, :])
            for half in range(2):
                cs = slice(1024 * half, 1024 * (half + 1))
                nc.sync.dma_start(out=ropeC[:, cs], in_=ropeC_d[:, cs])
                nc.sync.dma_start(out=ropeS[:, cs], in_=ropeS_d[:, cs])
                xT_chunk(1 + half)
            xT_chunk(3)

            V = vpool.tile([128, NKT, HPC * DK], BF16)

            def v_unit(t):
                def emit():
                    psv = ps.tile([128, 512], F32, tag="proj")
                    for d in range(ND):
                        nc.tensor.matmul(
                            psv[:, :],
                            xT[:, d, 128 * t : 128 * (t + 1)],
                            wv_sb[:, d, :],
                            start=(d == 0),
                            stop=(d == ND - 1),
                        )
                    # on ACT: keeps the DVE free for the rope chains that
                    # gate the first attention iterations
                    nc.scalar.copy(V[:, t, :], psv[:, :])
                return emit

            # aT resident in SBUF (bf16): [hp, 128 rows] x S
            aT = apool.tile([128, NHP, S], BF16)

            # w_o (bf16) -- loaded late via a filler unit during hp2
            wo_sb = wopool.tile([128, NHP, D], BF16)

            def wo_dma_unit():
                def emit():
                    nc.sync.dma_start(
                        out=wo_sb[:, :, :],
                        in_=wo_d[:, :].rearrange("(d p) c -> p d c", p=128),
                    )
                return emit

            # ---- per head-pair: Q^T/K^T projection + rope --------------
            def proj_units(hp, QT, KT):
                units = []
                state = {}

                def dma_unit(w_d, wtag):
                    def emit():
                        wt = wpool.tile([128, ND, 128], BF16, tag=wtag)
                        nc.sync.dma_start(
                            out=wt[:, :, :],
                            in_=w_d[
                                :, 128 * hp : 128 * (hp + 1)
                            ].rearrange("(d p) c -> p d c", p=128),
                        )
                        state[wtag] = wt
                    return emit

                def tb_unit(wtag, OUT, tb):
                    def emit():
                        wt = state[wtag]
                        psq = ps.tile([128, 512], F32, tag="proj")
                        for d in range(ND):
                            nc.tensor.matmul(
                                psq[:, :],
                                wt[:, d, :],
                                xT[:, d, 512 * tb : 512 * (tb + 1)],
                                start=(d == 0),
                                stop=(d == ND - 1),
                            )
                        # rope: OUT = psq*C + swap32(psq)*S
                        t2 = tmp.tile([128, 512], F32, tag="t2")
                        cs = slice(512 * tb, 512 * (tb + 1))
                        for h2 in range(2):
                            b0 = 64 * h2
                            nc.vector.tensor_mul(
                                t2[b0 : b0 + 32, :],
                                psq[b0 + 32 : b0 + 64, :],
                                ropeS[b0 : b0 + 32, cs],
                            )
                            nc.vector.tensor_mul(
                                t2[b0 + 32 : b0 + 64, :],
                                psq[b0 : b0 + 32, :],
                                ropeS[b0 + 32 : b0 + 64, cs],
                            )
                        t1 = tmp.tile([128, 512], F32, tag="t1")
                        nc.vector.tensor_mul(t1[:, :], psq[:, :], ropeC[:, cs])
                        nc.gpsimd.tensor_add(OUT[:, cs], t1[:, :], t2[:, :])
                    return emit

                # (q, k) interleaved per token block so KT-tb0's rope is
                # 2nd (not 5th) in the DVE queue -- attention qb0 needs
                # both QT-tb0 and KT-tb0.
                units.append(dma_unit(wq_d, "wq"))
                units.append(dma_unit(wk_d, "wk"))
                for tb in range(NT):
                    units.append(tb_unit("wq", QT, tb))
                    units.append(tb_unit("wk", KT, tb))
                return units

            qk_tiles = []
            for hp in range(NHP):
                qt_tile = qkpool.tile([128, S], BF16, tag="qt")
                kt_tile = qkpool.tile([128, S], BF16, tag="kt")
                qk_tiles.append((qt_tile, kt_tile))

            # ---- o_proj units (per (et, tb)) ---------------------------
            def o_unit(et, tb):
                def emit():
                    psy = ps.tile([128, 512], F32, tag="proj")
                    for dd in range(NHP):
                        nc.tensor.matmul(
                            psy[:, :],
                            wo_sb[:, dd, 128 * et : 128 * (et + 1)],
                            aT[:, dd, 512 * tb : 512 * (tb + 1)],
                            start=(dd == 0),
                            stop=(dd == NHP - 1),
                        )
                    y_t = ypool.tile([128, 512], BF16, tag="y")
                    nc.vector.tensor_copy(y_t[:, :], psy[:, :])
                    nc.sync.dma_start(
                        out=yT_d[
                            128 * et : 128 * (et + 1),
                            512 * tb : 512 * (tb + 1),
                        ],
                        in_=y_t[:, :],
                    )
                return emit

            # ---- pre-phase: V[0:4], hp0 projection ---------------------
            for t in range(4):
                v_unit(t)()
            for emit in proj_units(0, *qk_tiles[0]):
                emit()

            # ---- attention loop over head pairs ------------------------
            for hp in range(NHP):
                QT, KT = qk_tiles[hp]
                pending = []
                if hp == 0:
                    # remaining V tiles + hp1 projection
                    nxt = proj_units(1, *qk_tiles[1])
                    vs = [v_unit(t) for t in range(4, NKT)]
                    # interleave: V first (needed sooner), then projection
                    for i in range(max(len(vs), len(nxt))):
                        if i < len(vs):
                            pending.append(vs[i])
                        if i < len(nxt):
                            pending.append(nxt[i])
                elif hp + 1 < NHP:
                    pending = list(proj_units(hp + 1, *qk_tiles[hp + 1]))
                    if hp == 2:
                        pending.append(wo_dma_unit())
                pending.reverse()  # pop() from the front
                # hp0: dense fillers (ramp is dependency-gated); later hps:
                # spread the ~11 units across all 40 iterations so the tail
                # iterations still have work to absorb the exp latency
                every = 1 if hp == 0 else 4
                slot = 0

                for qb in range(NT):
                    # per-head PSUM banks so the packed PV / denom matmul
                    # pairs can execute concurrently (same-bank pairs
                    # serialize on the PSUM write port).  Denominators live
                    # in po's unused partition rows of the OTHER head's
                    # bank: h0 -> bank1 row 0, h1 -> bank0 row 64.
                    po = pv.tile([128, 2, 512], F32, tag="pv")
                    nkb = 4 * qb + 4
                    qlo = 512 * qb

                    def emit_scores(kb):
                        pss = ps.tile([128, 2, 512], F32, tag="score")
                        r = kb - 4 * qb
                        q0s = max(128 * r, 0)
                        for h2 in range(2):
                            b0 = 64 * h2
                            nc.tensor.matmul(
                                pss[:, h2, q0s:512],
                                KT[b0 : b0 + 64, 128 * kb : 128 * (kb + 1)],
                                QT[b0 : b0 + 64, qlo + q0s : qlo + 512],
                                start=True,
                                stop=True,
                                tile_position=(b0, 0),
                                skip_group_check=True,
                            )
                        return pss

                    pss_cur = emit_scores(0)
                    for kb in range(nkb):
                        pss = pss_cur
                        if kb + 1 < nkb:
                            pss_cur = emit_scores(kb + 1)
                        slot += 1
                        if pending and slot % every == 0:
                            pending.pop()()
                        r = kb - 4 * qb
                        q0 = 128 * r if r >= 0 else 0
                        es_t = es.tile([128, 2, 512], BF16, tag="es")
                        nc.scalar.activation(
                            es_t[:, :, q0:512],
                            pss[:, :, q0:512],
                            mybir.ActivationFunctionType.Exp,
                        )
                        if r >= 0:
                            # zero the strict upper triangle of the
                            # diagonal 128x128 block (both heads at once)
                            nc.vector.tensor_mul(
                                es_t[:, :, q0 : q0 + 128],
                                es_t[:, :, q0 : q0 + 128],
                                mask01[:, :, :],
                            )
                        first = kb == 0
                        last = kb == nkb - 1
                        for h2 in range(2):
                            b0 = 64 * h2
                            h_global = 2 * hp + h2
                            nc.tensor.matmul(
                                po[b0 : b0 + 64, h2, q0:512],
                                V[:, kb, 64 * h_global : 64 * (h_global + 1)],
                                es_t[:, h2, q0:512],
                                start=first,
                                stop=last,
                                tile_position=(0, b0),
                                skip_group_check=True,
                            )
                        nc.tensor.matmul(
                            po[0:1, 1, q0:512],
                            ones_col[:, :],
                            es_t[:, 0, q0:512],
                            start=first,
                            stop=last,
                            tile_position=(0, 0),
                            skip_group_check=True,
                        )
                        nc.tensor.matmul(
                            po[64:65, 0, q0:512],
                            ones_col[:, :],
                            es_t[:, 1, q0:512],
                            start=first,
                            stop=last,
                            tile_position=(0, 64),
                            skip_group_check=True,
                        )

                    # ---- normalize: aT = po / denom --------------------
                    # single early copy of po to SBUF frees the po PSUM
                    # banks for the next query block's PV immediately; the
                    # rest of the chain runs off the SBUF copy.
                    poc = tmp.tile([128, 2, 512], BF16, tag="poc", bufs=1)
                    nc.vector.tensor_copy(poc[:, :, :], po[:, :, :])
                    # broadcast denom across the 64 head rows on the PE
                    psb = ps.tile([128, 512], F32, tag="proj")
                    nc.tensor.matmul(
                        psb[0:64, :],
                        ones_row[0:1, :],
                        poc[0:1, 1, :],
                        start=True,
                        stop=True,
                        tile_position=(0, 0),
                        skip_group_check=True,
                    )
                    nc.tensor.matmul(
                        psb[64:128, :],
                        ones_row[64:65, :],
                        poc[64:65, 0, :],
                        start=True,
                        stop=True,
                        tile_position=(64, 64),
                        skip_group_check=True,
                    )
                    recbc = tmp.tile([128, 512], F32, tag="rec", bufs=1)
                    nc.vector.reciprocal_approx_fast(recbc[:, :], psb[:, :])
                    nc.vector.tensor_mul(
                        aT[0:64, hp, qlo : qlo + 512],
                        poc[0:64, 0, :],
                        recbc[0:64, :],
                    )
                    nc.vector.tensor_mul(
                        aT[64:128, hp, qlo : qlo + 512],
                        poc[64:128, 1, :],
                        recbc[64:128, :],
                    )

                    # hp3: o_proj for token-block qb becomes available
                    if hp == NHP - 1:
                        if qb < NT - 1:
                            for et in reversed(range(ND)):
                                pending.append(o_unit(et, qb))
                            every = 1
                        else:
                            # flush: remaining interleaved + final tb
                            while pending:
                                pending.pop()()
                            for et in range(ND):
                                o_unit(et, NT - 1)()

                # flush leftover fillers at head-pair boundary
                while pending:
                    pending.pop()()

    nc.compile()
    return nc


_PERM = np.concatenate([np.arange(0, DK, 2), np.arange(1, DK, 2)])


def _prep_core_inputs(x, token_positions, w_qkv, w_o, core):
    from ml_dtypes import bfloat16

    b = core // 2
    h0 = HPC * (core % 2)

    xT = np.ascontiguousarray(x[b].T.astype(np.float32).astype(bfloat16))

    w_q = w_qkv[0 * D : 1 * D]
    w_k = w_qkv[1 * D : 2 * D]
    w_v = w_qkv[2 * D : 3 * D]

    def gather(w, permute, scale):
        rows = []
        for j in range(HPC):
            g = h0 + j
            blk = w[DK * g : DK * (g + 1)]
            if permute:
                blk = blk[_PERM]
            rows.append(blk)
        out = np.concatenate(rows, axis=0).astype(np.float32) * scale
        return np.ascontiguousarray(out.T.astype(bfloat16))  # [D, HPC*DK]

    wq = gather(w_q, True, 1.0 / math.sqrt(DK))
    wk = gather(w_k, True, 1.0)
    wv = gather(w_v, False, 1.0)

    # w_o: [e_out, d_in]; take the d rows of this core's heads -> [512, D]
    rows = []
    for j in range(HPC):
        g = h0 + j
        rows.append(w_o[:, DK * g : DK * (g + 1)].T)
    wo = np.ascontiguousarray(
        np.concatenate(rows, axis=0).astype(bfloat16)
    )

    pos = token_positions.astype(np.float32)
    inv = (10000.0 ** (-(np.arange(0, DK, 2, dtype=np.float32)) / DK)).astype(
        np.float32
    )
    ang = pos[:, None] * inv[None, :]  # [S, 32]
    c = np.cos(ang).T.astype(np.float32)  # [32, S]
    s = np.sin(ang).T.astype(np.float32)
    C64 = np.concatenate([c, c], axis=0)
    S64 = np.concatenate([-s, s], axis=0)
    ropeC = np.ascontiguousarray(
        np.concatenate([C64, C64], axis=0).astype(bfloat16)
    )
    ropeS = np.ascontiguousarray(
        np.concatenate([S64, S64], axis=0).astype(bfloat16)
    )

    ki = np.arange(128)[:, None]
    qi = np.arange(128)[None, :]
    m01 = np.where(ki <= qi, 1.0, 0.0).astype(bfloat16)
    mask = np.ascontiguousarray(
        np.broadcast_to(m01[:, None, :], (128, 2, 128))
    )

    return {
        "xT": xT,
        "wq": wq,
        "wk": wk,
        "wv": wv,
        "wo": wo,
        "ropeC": ropeC,
        "ropeS": ropeS,
        "mask": mask,
    }


def kernel(x, token_positions, w_qkv, w_o):
    x = np.asarray(x, dtype=np.float32)
    token_positions = np.asarray(token_positions)
    w_qkv = np.asarray(w_qkv, dtype=np.float32)
    w_o = np.asarray(w_o, dtype=np.float32)

    if "nc" not in _CACHE:
        _CACHE["nc"] = _build()
    nc = _CACHE["nc"]

    in_maps = [
        _prep_core_inputs(x, token_positions, w_qkv, w_o, c)
        for c in range(NCORES)
    ]
    res = run_bass_kernel_spmd(nc, in_maps, core_ids=list(range(NCORES)))
    _CACHE["last_results"] = res

    out = np.empty((B, S, D), dtype=np.float32)
    for b in range(B):
        yT = res.results[2 * b]["yT"].astype(np.float32) + res.results[
            2 * b + 1
        ]["yT"].astype(np.float32)
        out[b] = yT.T
    return out
